# revision 42
# baseline (speedup 1.0000x reference)
"""CRF mean-field (nn_CRF) Trainium2 kernel, SPMD over 8 NeuronCores.

Math: 5 iterations of
    p   = softmax(q, axis=classes)
    out_f = p @ K_f        for two Gaussian kernels K_f (spatial, bilateral)
    q   = unaries - compat @ (sw @ out_sp + bw @ out_bl)

Design (v2, rewritten from the slab-streaming baseline):

  * SPATIAL filter: theta_gamma=8 makes the spatial kernel near-degenerate
    (exponent |u| <= ~0.4), so exp(s_i.s_j) is replaced by an EXACT degree-4
    polynomial feature map: K_sp ~= Psi^T Psi with Psi [35, N]
    (monomials * exp(-|s|^2/2), max elementwise error 2.8e-5). Per iteration
    this is two tiny PE matmuls (W = Psi p^T, out_sp = W^T Psi_loc) instead of
    half the N x N slab.

  * BILATERAL filter: dense [N, NL] fp8e4 slab, FULLY SBUF-resident
    (64 KB/partition), built once in iteration 0. The PE computes
    y = C1*G + 2^30 (G = f_i.f_j - |f_i|^2/2 - |f_j|^2/2, C1 = 2^23/ln2)
    via an augmented 9-row bf16 matmul; tiles alternate between
    ScalarE exp (activation with scale=1/C1) and a DVE Schraudolph exp
    (round y to int32, bitcast to f32 => 2^(y/2^23 - 127) ~ 2*gamma*e^G).
    The bf16 rounding of the j-side |f_j|^2 row is corrected EXACTLY by a
    per-point factor afix folded into the q update; the Schraudolph mean
    ratio gamma and the 2x scale are folded into amT_bl on the host.

  * MAIN matmul: p8 [128,2,10] fp8 stationary x slab [128,2,1024] fp8 moving,
    perf_mode=DoubleRow (256-deep contraction per instruction), spread over
    4 PE column groups (tile_position (0,0|32|64|96)) that stream
    concurrently: 8192 PE cycles per iteration. 4 partial sums are combined
    by DVE straight out of PSUM.

  * Per iteration cores exchange their local p shard (20KB bf16) via
    AllGather; iteration-0 softmax is computed on the host.
"""

import numpy as np
import ml_dtypes

C = 10          # classes
N = 8192        # points
S = 3           # spatial dims
R = 8           # cores
NL = N // R     # local points per core
KCH = N // 128  # 64 i-chunks
KP = KCH // 2   # 32 i-chunk pairs (DoubleRow)
JCH = NL // 128  # 8 local j-chunks
NITER = 5
THETA_GAMMA = 8.0
DEG = 4         # spatial poly degree
M = 35          # monomials for DEG=4 in 3 vars
NGRP = 4        # PE column groups for the main matmul (plain fp8, 32-aligned)

C1 = float(2**23) / float(np.log(2.0))
C2A = float(2**30)
GAMMA = 1.0406829  # E[(1+r)/2^r], r~U[0,1): Schraudolph mean ratio

_CACHE = {}


def _build_program():
    import concourse.mybir as mybir
    import concourse.tile as tile
    from concourse import bacc
    from concourse.bass import ts, ds

    f32 = mybir.dt.float32
    bf16 = mybir.dt.bfloat16
    fp8 = mybir.dt.float8e4
    i32 = mybir.dt.int32
    EXP = mybir.ActivationFunctionType.Exp
    DR = mybir.MatmulPerfMode.DoubleRow

    nc = bacc.Bacc("TRN2", target_bir_lowering=False, debug=False, num_devices=R)

    # ---- I/O ---- (all host-side pre-transposed into [128, X] row-contiguous
    # layouts so every DMA moves large runs, not 20-byte scatters)
    ffa = nc.dram_tensor("ffa", [42, N], bf16, kind="ExternalInput")
    fla = nc.dram_tensor("fla", [42, NL], bf16, kind="ExternalInput")
    psiT = nc.dram_tensor("psiT", [128, KCH, M], bf16, kind="ExternalInput")
    psil = nc.dram_tensor("psil", [M, NL], bf16, kind="ExternalInput")
    p_init = nc.dram_tensor("p_init", [128, KCH, C], bf16, kind="ExternalInput")
    unT_loc = nc.dram_tensor("unT_loc", [128, JCH, C], f32, kind="ExternalInput")
    amT_sp = nc.dram_tensor("amT_sp", [C, C], bf16, kind="ExternalInput")
    amT_bl = nc.dram_tensor("amT_bl", [C, C], bf16, kind="ExternalInput")
    qT_out = nc.dram_tensor("qT_out", [128, JCH, C], f32, kind="ExternalOutput")

    b_act = -C2A / C1 + float(np.log(2.0)) + float(np.log(GAMMA))

    with tile.TileContext(nc) as tc:
        with (
            tc.tile_pool(name="const", bufs=1) as const,
            tc.tile_pool(name="state", bufs=1) as state,
            tc.tile_pool(name="scpool", bufs=2) as scpool,
            tc.tile_pool(name="opool", bufs=2) as opool,
            tc.tile_pool(name="qpool", bufs=2) as qpool,
            tc.tile_pool(name="psO", bufs=1, space="PSUM") as psO,
            tc.tile_pool(name="psA", bufs=3, space="PSUM") as psA,
            tc.tile_pool(name="dram", bufs=2, space="DRAM") as dram,
        ):
            # ---- constants (loads split across engine DMA queues) ----
            ffa_sb = const.tile([42, N], bf16, name="ffa_sb")
            fla_sb = const.tile([42, NL], bf16, name="fla_sb")
            HN = N // 2
            nc.sync.dma_start(fla_sb[:], fla[:])
            nc.sync.dma_start(ffa_sb[0:10, 0:HN], ffa[0:10, 0:HN])
            nc.scalar.dma_start(ffa_sb[32:42, 0:HN], ffa[32:42, 0:HN])
            nc.sync.dma_start(ffa_sb[0:10, HN:N], ffa[0:10, HN:N])
            nc.scalar.dma_start(ffa_sb[32:42, HN:N], ffa[32:42, HN:N])
            psiT_sb = const.tile([128, KCH, M], bf16, name="psiT_sb")
            psil_sb = const.tile([M, NL], bf16, name="psil_sb")
            amT_sp_sb = const.tile([C, C], bf16, name="amT_sp_sb")
            amT_bl_sb = const.tile([C, C], bf16, name="amT_bl_sb")
            un_sb = const.tile([128, JCH, C], f32, name="un_sb")
            nc.gpsimd.dma_start(psiT_sb[:, 0:KCH // 2, :], psiT[:, 0:KCH // 2, :])
            nc.gpsimd.dma_start(psiT_sb[:, KCH // 2:, :], psiT[:, KCH // 2:, :])
            nc.gpsimd.dma_start(psil_sb[:], psil[:])
            nc.gpsimd.dma_start(amT_sp_sb[:], amT_sp[:])
            nc.gpsimd.dma_start(amT_bl_sb[:], amT_bl[:])
            nc.gpsimd.dma_start(un_sb[:], unT_loc[:])

            # bilateral slab, fp8, SBUF-resident
            slab = const.tile([128, KCH, NL], fp8, name="slab")

            bact_sb = const.tile([128, 1], f32, name="bact_sb")
            nc.gpsimd.memset(bact_sb[:], b_act)

            # class distribution: bf16 and fp8-packed copies
            p_sb = state.tile([128, KCH, C], bf16, name="p_sb")
            p8_sb = state.tile([128, KCH, C], fp8, name="p8_sb")
            nc.sync.dma_start(p_sb[:], p_init[:])
            nc.scalar.copy(p8_sb[:], p_sb[:])

            # softmax scratch
            mx_sb = state.tile([128, JCH], f32, name="mx_sb")
            sm_sb = state.tile([128, JCH], f32, name="sm_sb")
            rs_sb = state.tile([128, JCH], f32, name="rs_sb")
            el_sb = state.tile([128, JCH, C], f32, name="el_sb")
            wt_hi = state.tile([M, C], bf16, name="wt_hi")
            wt_lo = state.tile([M, C], bf16, name="wt_lo")

            for t in range(NITER):
                if t == 0:
                    # ---- build the fp8 slab: 64 tiles of [128, NL] ----
                    # ScalarE exp : DVE Schraudolph at 2:1 (measured rates)
                    for k in range(KCH):
                        rs = 32 * (k % 2)
                        yt = psA.tile([128, NL], f32, name="yt", tag="A")
                        for jh in range(2):
                            jsl = ds(jh * 512, 512)
                            nc.tensor.matmul(
                                yt[:, jsl],
                                ffa_sb[rs:rs + 10, ts(k, 128)],
                                fla_sb[rs:rs + 10, jsl],
                                start=True, stop=True,
                                tile_position=(rs, 0),
                            )
                        if k % 3 != 2:
                            nc.scalar.activation(
                                slab[:, k, :], yt[:], EXP,
                                bias=bact_sb[:], scale=1.0 / C1,
                            )
                        else:
                            sc = scpool.tile([128, NL], i32, name="sc")
                            nc.vector.tensor_scalar_add(sc[:], yt[:], 0.0)
                            nc.vector.tensor_scalar_add(
                                slab[:, k, :], sc[:].bitcast(f32), 0.0
                            )

                # ---- spatial W^T (fast, scheduled right after p arrives) ----
                wtp = psA.tile([M, C], f32, name="wtp", tag="A")
                for k in range(KCH):
                    nc.tensor.matmul(
                        wtp[:], psiT_sb[:, k, :], p_sb[:, k, :],
                        start=(k == 0), stop=(k == KCH - 1),
                    )
                nc.scalar.copy(wt_hi[:], wtp[:])
                nc.vector.tensor_sub(wt_lo[:], wtp[:], wt_hi[:])
                osp = psA.tile([C, NL], f32, name="osp", tag="A")

                # ---- mains j-half-major; each half's post-processing overlaps
                # the other half's matmuls (different PSUM banks) ----
                po = psO.tile([128, NL], f32, name="po")
                s1 = opool.tile([C, NL], f32, name="s1")
                c01 = opool.tile([C, NL], f32, name="c01")
                c013 = opool.tile([C, NL], f32, name="c013")
                ot_b = opool.tile([C, NL], bf16, name="ot_b")
                ot_s = opool.tile([C, NL], bf16, name="ot_s")
                qa = psA.tile([128, JCH, C], f32, name="qa", tag="A")
                ql = qpool.tile([128, JCH, C], f32, name="ql")
                for jh in range(2):
                    jsl = ds(jh * 512, 512)
                    for k in range(KCH):
                        g = k % NGRP
                        nc.tensor.matmul(
                            po[32 * g:32 * g + C, jsl],
                            p8_sb[:, k, :],
                            slab[:, k, jsl],
                            tile_position=(0, 32 * g),
                            start=(k < NGRP), stop=(k >= KCH - NGRP),
                        )
                    nc.tensor.matmul(
                        osp[:, jsl], wt_hi[:], psil_sb[:, jsl],
                        start=True, stop=False,
                    )
                    nc.tensor.matmul(
                        osp[:, jsl], wt_lo[:], psil_sb[:, jsl],
                        start=False, stop=True,
                    )
                    # combine bl partials (<=1 PSUM operand per DVE op)
                    nc.scalar.copy(s1[:, jsl], po[32:32 + C, jsl])
                    nc.vector.tensor_add(c01[:, jsl], po[0:C, jsl], s1[:, jsl])
                    nc.vector.tensor_add(
                        c013[:, jsl], po[64:64 + C, jsl], c01[:, jsl]
                    )
                    nc.vector.tensor_add(
                        ot_b[:, jsl], po[96:96 + C, jsl], c013[:, jsl]
                    )
                    nc.scalar.copy(ot_s[:, jsl], osp[:, jsl])
                    for j in range(4 * jh, 4 * jh + 4):
                        nc.tensor.matmul(
                            qa[:, j, :], ot_b[:, ts(j, 128)], amT_bl_sb[:],
                            start=True, stop=False,
                        )
                        nc.tensor.matmul(
                            qa[:, j, :], ot_s[:, ts(j, 128)], amT_sp_sb[:],
                            start=False, stop=True,
                        )
                    jr = ds(4 * jh, 4)
                    nc.vector.tensor_add(
                        ql[:, jr, :], qa[:, jr, :], un_sb[:, jr, :]
                    )

                if t < NITER - 1:
                    # ---- per-half softmax -> p shard (bf16) -> AllGather ----
                    pl = qpool.tile([128, JCH, C], bf16, name="pl")
                    bi = dram.tile([128, JCH * C], bf16, name="bi")
                    bo = dram.tile([R, 128, JCH * C], bf16, addr_space="Shared",
                                   name="bo")
                    bi3 = bi[:].rearrange("p (j c) -> p j c", c=C)
                    for jh in range(2):
                        jr = ds(4 * jh, 4)
                        nc.vector.reduce_max(
                            mx_sb[:, jr], ql[:, jr, :], axis=mybir.AxisListType.X
                        )
                        mx_b = mx_sb[:, jr].unsqueeze(2).broadcast_to((128, 4, C))
                        nc.vector.tensor_sub(el_sb[:, jr, :], ql[:, jr, :], mx_b)
                        nc.scalar.activation(
                            el_sb[:, jr, :], el_sb[:, jr, :], EXP,
                            bias=0.0, scale=1.0,
                        )
                        nc.vector.reduce_sum(
                            sm_sb[:, jr], el_sb[:, jr, :], axis=mybir.AxisListType.X
                        )
                        nc.vector.reciprocal(rs_sb[:, jr], sm_sb[:, jr])
                        rs_b = rs_sb[:, jr].unsqueeze(2).broadcast_to((128, 4, C))
                        nc.vector.tensor_mul(pl[:, jr, :], el_sb[:, jr, :], rs_b)
                        eng = nc.sync if jh == 0 else nc.scalar
                        eng.dma_start(bi3[:, jr, :], pl[:, jr, :])
                    nc.gpsimd.collective_compute(
                        "AllGather",
                        mybir.AluOpType.bypass,
                        replica_groups=[list(range(R))],
                        ins=[bi[:].opt()],
                        outs=[bo[:].opt()],
                    )
                    p_sb4 = p_sb[:].rearrange("p (r y) c -> p r y c", r=R)
                    bo4 = bo[:].rearrange("r p (y c) -> p r y c", c=C)
                    nc.sync.dma_start(p_sb4[:, 0:4], bo4[:, 0:4])
                    nc.scalar.dma_start(p_sb4[:, 4:8], bo4[:, 4:8])
                    nc.scalar.copy(p8_sb[:], p_sb[:])
                else:
                    nc.sync.dma_start(qT_out[:], ql[:])

    nc.compile()
    return nc


def _get_program():
    if "nc" not in _CACHE:
        _CACHE["nc"] = _build_program()
    return _CACHE["nc"]


def _host_prep(unaries, feat, sw, bw, compat):
    bf = ml_dtypes.bfloat16
    f = feat.astype(np.float32)
    f2 = np.sum(f * f, axis=0)

    sqc = np.float32(np.sqrt(C1))
    fr = (sqc * f).astype(bf)                      # [6, N] bf16 scaled features
    r_row = (np.float32(C1) * (-0.5 * f2)).astype(bf)   # bf16 |f|^2 row

    # exact correction for the bf16 rounding of the j-side row, folded
    # into the exponent as one extra augmented row: v = C1*ln(afix)
    r_used = r_row.astype(np.float32)
    v_row = (r_used + np.float32(C1) * (0.5 * f2).astype(np.float32)).astype(bf)

    # i-side rows (lhsT): [sq*f(6); r_i; 1; 1; 1],
    # j-side rows (rhs):  [sq*f(6); 1; r_j; 2^30; v]
    ffa = np.zeros((42, N), dtype=bf)
    fla_full = np.zeros((42, N), dtype=bf)
    for off in (0, 32):
        ffa[off:off + 6] = fr
        ffa[off + 6] = r_row
        ffa[off + 7] = bf(1.0)
        ffa[off + 8] = bf(1.0)
        ffa[off + 9] = bf(1.0)
        fla_full[off:off + 6] = fr
        fla_full[off + 6] = bf(1.0)
        fla_full[off + 7] = r_row
        fla_full[off + 8] = bf(C2A)
        fla_full[off + 9] = v_row

    # spatial poly features
    from math import factorial
    s = f[:S] / np.float32(THETA_GAMMA)
    a_sp = np.exp(-0.5 * np.sum(s * s, axis=0))
    rows = []
    for a in range(DEG + 1):
        for b in range(DEG + 1 - a):
            for c in range(DEG + 1 - a - b):
                coef = 1.0 / np.sqrt(factorial(a) * factorial(b) * factorial(c))
                rows.append(coef * s[0] ** a * s[1] ** b * s[2] ** c)
    psi = (np.stack(rows) * a_sp[None, :]).astype(bf)    # [M, N]
    # [128, KCH, M]: psiT[p, k, m] = psi[m, 128k+p]
    psiT = np.ascontiguousarray(
        psi.T.reshape(KCH, 128, M).transpose(1, 0, 2)
    )

    amT_sp = np.ascontiguousarray((-(compat @ sw)).T).astype(bf)
    amT_bl = (np.ascontiguousarray((-(compat @ bw)).T)
              / np.float32(2.0 * GAMMA)).astype(bf)

    qT_init = np.ascontiguousarray(unaries.T).astype(np.float32)
    mx = unaries.max(axis=0, keepdims=True)
    e = np.exp(unaries - mx, dtype=np.float32)
    p0 = (e / e.sum(axis=0, keepdims=True)).astype(bf)
    # [128, KCH, C]: p0T[p, k, c] = p0[c, 128k+p]
    p0T = np.ascontiguousarray(p0.T.reshape(KCH, 128, C).transpose(1, 0, 2))
    return ffa, fla_full, psiT, psi, amT_sp, amT_bl, qT_init, p0T


def _make_in_maps(inputs):
    unaries = np.asarray(inputs["unaries"], dtype=np.float32)
    feat = np.asarray(inputs["feat"], dtype=np.float32)
    sw = np.asarray(inputs["spatial_weights"], dtype=np.float32)
    bw = np.asarray(inputs["bilateral_weights"], dtype=np.float32)
    compat = np.asarray(inputs["compatibility_matrix"], dtype=np.float32)

    ffa, fla_full, psiT, psi, amT_sp, amT_bl, qT_init, p0T = _host_prep(
        unaries, feat, sw, bw, compat
    )
    in_maps = []
    for r in range(R):
        jsl = slice(r * NL, (r + 1) * NL)
        in_maps.append({
            "ffa": ffa,
            "fla": np.ascontiguousarray(fla_full[:, jsl]),
            "psiT": psiT,
            "psil": np.ascontiguousarray(psi[:, jsl]),
            "p_init": p0T,
            "unT_loc": np.ascontiguousarray(
                qT_init[jsl].reshape(JCH, 128, C).transpose(1, 0, 2)
            ),
            "amT_sp": amT_sp,
            "amT_bl": amT_bl,
        })
    return in_maps


def kernel(unaries, feat, spatial_weights, bilateral_weights, compatibility_matrix):
    from concourse.bass_utils import run_bass_kernel_spmd

    in_maps = _make_in_maps({
        "unaries": unaries,
        "feat": feat,
        "spatial_weights": spatial_weights,
        "bilateral_weights": bilateral_weights,
        "compatibility_matrix": compatibility_matrix,
    })
    nc = _get_program()
    res = run_bass_kernel_spmd(nc, in_maps, core_ids=list(range(R)))

    q = np.empty((C, N), dtype=np.float32)
    for r in range(R):
        out = res.results[r]["qT_out"]          # [128, JCH, C]
        q[:, r * NL:(r + 1) * NL] = out.transpose(2, 1, 0).reshape(C, NL)
    return q
